# revision 20
# baseline (speedup 1.0000x reference)
"""Trainium2 Bass kernel for nn_CLoss_68521908241007 (retrieval_knn).

Math (per the reference):
  sq_dist[i,j] = ||feat_i||^2 + ||feat2_j||^2 - 2 feat_i . feat2_j
  logits = -temp * sqrt(sq_dist)
  loss = mean_i( logsumexp_j(logits[i,:]) - logits[i, labels_i] )

Sharding: feat rows split across 8 cores (1024 queries each); feat2 replicated.
Each core computes its 1024x8192 block and returns per-row losses; the host
concatenates and takes the mean (the "all-reduce").

Per-core device pipeline:
  - PE: psum = -0.5*y_sq (K=1 fold matmul) accumulated with featT.T @ feat2T
    (float32r, full rate), so psum = 0.5*(y_sq) ... actually psum holds
    (-0.5*y_sq + G); ACT applies scale=-2 giving (y_sq - 2G) and adds the
    per-partition bias x_sq inside the same op:
  - ACT: dist = Sqrt(-2*psum + x_sq)   [one pass, PSUM->SBUF, bf16 out]
         e    = Exp(-temp*dist)        [one pass, with fused row-sum accum]
  - ACT passes are batched by activation-table set (sqrt vs exp) to avoid
    ~2.7us table reloads per switch.
  - y_sq computed on device via DVE square + PE ones-reduction; x_sq and the
    picked-label distance via DVE tensor_tensor_reduce.
"""

import numpy as np
from contextlib import ExitStack

import concourse.bass as bass
import concourse.bacc as bacc
import concourse.mybir as mybir
import concourse.tile as tile
from concourse.bass_utils import run_bass_kernel_spmd

AF = mybir.ActivationFunctionType
ALU = mybir.AluOpType
f32 = mybir.dt.float32
f32r = mybir.dt.float32r
bf16 = mybir.dt.bfloat16

N_CORES = 8
N, M, D = 8192, 8192, 128
NQ = N // N_CORES        # queries per core
QB = NQ // 128           # q-blocks per core (8)
KSEG = 512               # keys per matmul
NKSEG = M // KSEG        # 16
GRP = 4                  # k-segs per psum group (4 banks)
NGRP = NKSEG // GRP      # 4 groups per q-block
BATCH = 4                # q-blocks per sqrt/exp table phase
assert QB % BATCH == 0
NPHASE = QB // BATCH


def _body(tc, out_d, featT_d, featn_d, feat2T_d, sel_d, temp_d):
    nc = tc.nc
    with ExitStack() as ctx:
        singles = ctx.enter_context(tc.tile_pool(name="singles", bufs=1))
        sqp = ctx.enter_context(tc.tile_pool(name="sqp", bufs=1))
        distp = ctx.enter_context(tc.tile_pool(name="distp", bufs=BATCH))
        psp = ctx.enter_context(tc.tile_pool(name="psp", bufs=2, space="PSUM"))
        smallp = ctx.enter_context(tc.tile_pool(name="smallp", bufs=2))

        # ---- inputs -> SBUF (matmul operands live as float32r)
        feat2T_sb = singles.tile([D, M], f32r)
        for s in range(NKSEG):
            nc.sync.dma_start(out=feat2T_sb[:, s * KSEG:(s + 1) * KSEG],
                              in_=feat2T_d[:, s * KSEG:(s + 1) * KSEG])
        featT_sb = singles.tile([D, NQ], f32r)
        nc.sync.dma_start(out=featT_sb, in_=featT_d)
        featn_sb = singles.tile([128, QB, D], f32)
        nc.sync.dma_start(out=featn_sb,
                          in_=featn_d.rearrange("(b p) d -> p b d", p=128))
        sel_sb = singles.tile([128, QB, D], f32)
        nc.sync.dma_start(out=sel_sb,
                          in_=sel_d.rearrange("(b p) d -> p b d", p=128))
        # temp broadcast to all 128 partitions straight from DRAM
        pos_temp = singles.tile([128, 1], f32)
        nc.sync.dma_start(out=pos_temp, in_=temp_d.to_broadcast((128, 1)))

        # ---- constants (ISA memset can't write f32r; memset f32 then DVE-copy)
        neghalf_f = singles.tile([1, 128], f32)
        nc.vector.memset(neghalf_f, -0.5)
        neghalf = singles.tile([1, 128], f32r)
        nc.vector.tensor_copy(neghalf, neghalf_f)
        ones_col_f = singles.tile([D, 1], f32)
        nc.vector.memset(ones_col_f, 1.0)
        ones_col = singles.tile([D, 1], f32r)
        nc.vector.tensor_copy(ones_col, ones_col_f)
        neg_temp = singles.tile([128, 1], f32)
        nc.vector.tensor_scalar_mul(neg_temp, pos_temp, -1.0)

        # ---- y_sq[1, M] = col norms of feat2T via DVE square + PE ones-reduce
        feat2T_f = feat2T_sb.bitcast(f32)
        y_sq = singles.tile([1, M], f32r)
        for s in range(NKSEG):
            sq = sqp.tile([128, KSEG], f32r, tag=f"sq{s}")
            nc.vector.tensor_mul(sq, feat2T_f[:, s * KSEG:(s + 1) * KSEG],
                                 feat2T_f[:, s * KSEG:(s + 1) * KSEG])
            ps_y = psp.tile([128, GRP * KSEG], f32, tag="ps")
            nc.tensor.matmul(ps_y[0:1, 0:KSEG], lhsT=ones_col,
                             rhs=sq, start=True, stop=True)
            nc.vector.tensor_copy(y_sq[0:1, s * KSEG:(s + 1) * KSEG],
                                  ps_y[0:1, 0:KSEG])

        # ---- x_sq[128, QB] = row norms of feat; psq = ||feat - feat2[label]||^2
        # (after the y_sq loop so DVE's vector clock already covers the DMA
        # queues -> tensor_sub needs at most one wait)
        x_sq = singles.tile([128, QB], f32)
        psq = singles.tile([128, QB], f32)
        # route sel through a DVE copy so the subtract carries at most one
        # cross-engine wait (TensorTensor has a single ISA wait slot)
        sel2 = singles.tile([128, QB, D], f32)
        nc.vector.tensor_copy(sel2, sel_sb)
        diff_all = singles.tile([128, QB, D], f32)
        nc.vector.tensor_sub(diff_all, featn_sb, sel2)
        for b in range(QB):
            scr = smallp.tile([128, D], f32, tag="scr")
            nc.scalar.activation(out=scr, in_=featn_sb[:, b, :],
                                 func=AF.Square, bias=0.0, scale=1.0,
                                 accum_out=x_sq[:, b:b + 1])
            scr2 = smallp.tile([128, D], f32, tag="scr")
            nc.scalar.activation(out=scr2, in_=diff_all[:, b, :],
                                 func=AF.Square, bias=0.0, scale=1.0,
                                 accum_out=psq[:, b:b + 1])

        feat2T_r = feat2T_sb
        featT_r = featT_sb
        y_sq_r = y_sq

        # ---- main: NPHASE phases of BATCH q-blocks (sqrt batch, then exp batch)
        S = singles.tile([128, QB], f32)
        pdist = singles.tile([128, QB], f32)
        for ph in range(NPHASE):
            qbs = range(ph * BATCH, (ph + 1) * BATCH)
            dist_tiles = {}
            for b in qbs:
                dist_t = distp.tile([128, M], bf16, tag="dist")
                dist_tiles[b] = dist_t
                for g in range(NGRP):
                    ps = psp.tile([128, GRP * KSEG], f32, tag="ps")
                    for si in range(GRP):
                        s = g * GRP + si
                        sl = ps[:, si * KSEG:(si + 1) * KSEG]
                        nc.tensor.matmul(
                            sl, lhsT=neghalf,
                            rhs=y_sq_r[0:1, s * KSEG:(s + 1) * KSEG],
                            start=True, stop=False)
                        nc.tensor.matmul(
                            sl, lhsT=featT_r[:, b * 128:(b + 1) * 128],
                            rhs=feat2T_r[:, s * KSEG:(s + 1) * KSEG],
                            start=False, stop=True)
                    nc.scalar.activation(
                        out=dist_t[:, g * GRP * KSEG:(g + 1) * GRP * KSEG],
                        in_=ps, func=AF.Sqrt,
                        bias=x_sq[:, b:b + 1], scale=-2.0)
            if ph == NPHASE - 1:
                # picked-label distance; still in the sqrt table phase
                nc.scalar.activation(out=pdist, in_=psq, func=AF.Sqrt,
                                     bias=0.0, scale=1.0)
            for b in qbs:
                dist_t = dist_tiles.pop(b)
                nc.scalar.activation(
                    out=dist_t, in_=dist_t, func=AF.Exp,
                    bias=0.0, scale=neg_temp[:, 0:1],
                    accum_out=S[:, b:b + 1])

        # ---- finals: loss_row = Ln(S) + temp * pdist
        logz = singles.tile([128, QB], f32)
        nc.scalar.activation(out=logz, in_=S, func=AF.Ln, bias=0.0, scale=1.0)
        picked = singles.tile([128, QB], f32)
        nc.vector.tensor_scalar_mul(picked, pdist, pos_temp[:, 0:1])
        loss_t = singles.tile([128, QB], f32)
        nc.vector.tensor_add(loss_t, picked, logz)
        nc.sync.dma_start(out=out_d, in_=loss_t)


def build_program():
    nc = bacc.Bacc("TRN2", target_bir_lowering=False, debug=False,
                   num_devices=N_CORES)
    featT = nc.dram_tensor("featT", [D, NQ], f32r, kind="ExternalInput").ap()
    featn = nc.dram_tensor("featn", [NQ, D], f32, kind="ExternalInput").ap()
    feat2T = nc.dram_tensor("feat2T", [D, M], f32r, kind="ExternalInput").ap()
    sel = nc.dram_tensor("sel", [NQ, D], f32, kind="ExternalInput").ap()
    temp = nc.dram_tensor("temp", [1, 1], f32, kind="ExternalInput").ap()
    out = nc.dram_tensor("out", [128, QB], f32, kind="ExternalOutput").ap()
    with tile.TileContext(nc) as tc:
        _body(tc, out, featT, featn, feat2T, sel, temp)
    nc.compile()
    return nc


def make_in_maps(feat, feat2, temp, labels):
    feat = np.ascontiguousarray(np.asarray(feat, dtype=np.float32))
    feat2 = np.ascontiguousarray(np.asarray(feat2, dtype=np.float32))
    labels_np = np.asarray(labels).astype(np.int64)
    temp_np = np.asarray(temp, dtype=np.float32).reshape(1, 1)
    feat2T = np.ascontiguousarray(feat2.T)
    sel_full = feat2[labels_np]
    in_maps = []
    for c in range(N_CORES):
        fs = feat[c * NQ:(c + 1) * NQ]
        in_maps.append({
            "featT": np.ascontiguousarray(fs.T),
            "featn": fs,
            "feat2T": feat2T,
            "sel": np.ascontiguousarray(sel_full[c * NQ:(c + 1) * NQ]),
            "temp": temp_np,
        })
    return in_maps


def combine_outputs(per_core_outs):
    # out[p, b] is the loss for query q = b*128 + p of that core's shard
    rows = [np.asarray(o).T.reshape(-1) for o in per_core_outs]
    return np.float32(np.concatenate(rows).mean())


_PROGRAM = None


def kernel(feat, feat2, temp, labels):
    global _PROGRAM
    if _PROGRAM is None:
        _PROGRAM = build_program()
    in_maps = make_in_maps(feat, feat2, temp, labels)
    res = run_bass_kernel_spmd(_PROGRAM, in_maps, core_ids=list(range(N_CORES)))
    return combine_outputs([r["out"] for r in res.results])


# revision 21
# speedup vs baseline: 1.1075x; 1.1075x over previous
"""Trainium2 Bass kernel for nn_CLoss_68521908241007 (retrieval_knn).

Math (per the reference):
  sq_dist[i,j] = ||feat_i||^2 + ||feat2_j||^2 - 2 feat_i . feat2_j
  logits = -temp * sqrt(sq_dist)
  loss = mean_i( logsumexp_j(logits[i,:]) - logits[i, labels_i] )

Sharding: feat rows split across 8 cores (1024 queries each); feat2 replicated.
Each core computes its 1024x8192 block and returns per-row losses; the host
concatenates and takes the mean (the "all-reduce").

Per-core device pipeline:
  - PE: psum = -0.5*y_sq (K=1 fold matmul) accumulated with featT.T @ feat2T
    (float32r, full rate), so psum = 0.5*(y_sq) ... actually psum holds
    (-0.5*y_sq + G); ACT applies scale=-2 giving (y_sq - 2G) and adds the
    per-partition bias x_sq inside the same op:
  - ACT: dist = Sqrt(-2*psum + x_sq)   [one pass, PSUM->SBUF, bf16 out]
         e    = Exp(-temp*dist)        [one pass, with fused row-sum accum]
  - ACT passes are batched by activation-table set (sqrt vs exp) to avoid
    ~2.7us table reloads per switch.
  - y_sq computed on device via DVE square + PE ones-reduction; x_sq and the
    picked-label distance via DVE tensor_tensor_reduce.
"""

import numpy as np
from contextlib import ExitStack

import concourse.bass as bass
import concourse.bacc as bacc
import concourse.mybir as mybir
import concourse.tile as tile
from concourse.bass_utils import run_bass_kernel_spmd

AF = mybir.ActivationFunctionType
ALU = mybir.AluOpType
f32 = mybir.dt.float32
f32r = mybir.dt.float32r
bf16 = mybir.dt.bfloat16

N_CORES = 8
N, M, D = 8192, 8192, 128
NQ = N // N_CORES        # queries per core
QB = NQ // 128           # q-blocks per core (8)
KSEG = 512               # keys per matmul
NKSEG = M // KSEG        # 16
GRP = 4                  # k-segs per psum group (4 banks)
NGRP = NKSEG // GRP      # 4 groups per q-block
BATCH = 4                # q-blocks per sqrt/exp table phase
assert QB % BATCH == 0
NPHASE = QB // BATCH


def _body(tc, out_d, featT_d, featn_d, feat2T_d, sel_d, temp_d):
    nc = tc.nc
    with ExitStack() as ctx:
        singles = ctx.enter_context(tc.tile_pool(name="singles", bufs=1))
        sqp = ctx.enter_context(tc.tile_pool(name="sqp", bufs=1))
        distp = ctx.enter_context(tc.tile_pool(name="distp", bufs=BATCH))
        psp = ctx.enter_context(tc.tile_pool(name="psp", bufs=2, space="PSUM"))
        smallp = ctx.enter_context(tc.tile_pool(name="smallp", bufs=2))

        # ---- inputs -> SBUF (matmul operands live as float32r)
        feat2T_sb = singles.tile([D, M], bf16)
        for s in range(NKSEG):
            nc.sync.dma_start(out=feat2T_sb[:, s * KSEG:(s + 1) * KSEG],
                              in_=feat2T_d[:, s * KSEG:(s + 1) * KSEG])
        featT_sb = singles.tile([D, NQ], bf16)
        nc.sync.dma_start(out=featT_sb, in_=featT_d)
        featn_sb = singles.tile([128, QB, D], f32)
        nc.sync.dma_start(out=featn_sb,
                          in_=featn_d.rearrange("(b p) d -> p b d", p=128))
        sel_sb = singles.tile([128, QB, D], f32)
        nc.sync.dma_start(out=sel_sb,
                          in_=sel_d.rearrange("(b p) d -> p b d", p=128))
        # temp broadcast to all 128 partitions straight from DRAM
        pos_temp = singles.tile([128, 1], f32)
        nc.sync.dma_start(out=pos_temp, in_=temp_d.to_broadcast((128, 1)))

        # ---- constants (ISA memset can't write f32r; memset f32 then DVE-copy)
        neghalf_f = singles.tile([1, 128], f32)
        nc.vector.memset(neghalf_f, -0.5)
        neghalf = singles.tile([1, 128], bf16)
        nc.vector.tensor_copy(neghalf, neghalf_f)
        ones_col_f = singles.tile([D, 1], f32)
        nc.vector.memset(ones_col_f, 1.0)
        ones_col = singles.tile([D, 1], bf16)
        nc.vector.tensor_copy(ones_col, ones_col_f)
        neg_temp = singles.tile([128, 1], f32)
        nc.vector.tensor_scalar_mul(neg_temp, pos_temp, -1.0)

        # ---- y_sq[1, M] = col norms of feat2T via DVE square + PE ones-reduce
        # fold rhs is bf16(y_sq - 128): the mean shift keeps the bf16
        # rounding of the fold operand ~3x tighter
        y_sq = singles.tile([1, M], bf16)
        for s in range(NKSEG):
            sq = sqp.tile([128, KSEG], bf16, tag=f"sq{s}")
            nc.vector.tensor_mul(sq, feat2T_sb[:, s * KSEG:(s + 1) * KSEG],
                                 feat2T_sb[:, s * KSEG:(s + 1) * KSEG])
            ps_y = psp.tile([128, GRP * KSEG], f32, tag="ps")
            nc.tensor.matmul(ps_y[0:1, 0:KSEG], lhsT=ones_col,
                             rhs=sq, start=True, stop=True)
            nc.vector.tensor_scalar_add(y_sq[0:1, s * KSEG:(s + 1) * KSEG],
                                        ps_y[0:1, 0:KSEG], -128.0)

        # ---- x_sq[128, QB] = row norms of feat; psq = ||feat - feat2[label]||^2
        # (after the y_sq loop so DVE's vector clock already covers the DMA
        # queues -> tensor_sub needs at most one wait)
        x_sq = singles.tile([128, QB], f32)
        psq = singles.tile([128, QB], f32)
        # route sel through a DVE copy so the subtract carries at most one
        # cross-engine wait (TensorTensor has a single ISA wait slot)
        sel2 = singles.tile([128, QB, D], f32)
        nc.vector.tensor_copy(sel2, sel_sb)
        diff_all = singles.tile([128, QB, D], f32)
        nc.vector.tensor_sub(diff_all, featn_sb, sel2)
        for b in range(QB):
            scr = smallp.tile([128, D], f32, tag="scr")
            nc.scalar.activation(out=scr, in_=featn_sb[:, b, :],
                                 func=AF.Square, bias=0.0, scale=1.0,
                                 accum_out=x_sq[:, b:b + 1])
            scr2 = smallp.tile([128, D], f32, tag="scr")
            nc.scalar.activation(out=scr2, in_=diff_all[:, b, :],
                                 func=AF.Square, bias=0.0, scale=1.0,
                                 accum_out=psq[:, b:b + 1])

        xb = singles.tile([128, QB], f32)
        nc.vector.tensor_scalar_add(xb, x_sq, 128.0)

        # ---- main: NPHASE phases of BATCH q-blocks (sqrt batch, then exp batch)
        S = singles.tile([128, QB], f32)
        pdist = singles.tile([128, QB], f32)
        for ph in range(NPHASE):
            qbs = range(ph * BATCH, (ph + 1) * BATCH)
            dist_tiles = {}
            for b in qbs:
                dist_t = distp.tile([128, M], bf16, tag="dist")
                dist_tiles[b] = dist_t
                for g in range(NGRP):
                    ps = psp.tile([128, GRP * KSEG], f32, tag="ps")
                    for si in range(GRP):
                        s = g * GRP + si
                        sl = ps[:, si * KSEG:(si + 1) * KSEG]
                        nc.tensor.matmul(
                            sl, lhsT=neghalf,
                            rhs=y_sq[0:1, s * KSEG:(s + 1) * KSEG],
                            start=True, stop=False)
                        nc.tensor.matmul(
                            sl, lhsT=featT_sb[:, b * 128:(b + 1) * 128],
                            rhs=feat2T_sb[:, s * KSEG:(s + 1) * KSEG],
                            start=False, stop=True)
                    nc.scalar.activation(
                        out=dist_t[:, g * GRP * KSEG:(g + 1) * GRP * KSEG],
                        in_=ps, func=AF.Sqrt,
                        bias=xb[:, b:b + 1], scale=-2.0)
            if ph == NPHASE - 1:
                # picked-label distance; still in the sqrt table phase
                nc.scalar.activation(out=pdist, in_=psq, func=AF.Sqrt,
                                     bias=0.0, scale=1.0)
            for b in qbs:
                dist_t = dist_tiles.pop(b)
                nc.scalar.activation(
                    out=dist_t, in_=dist_t, func=AF.Exp,
                    bias=0.0, scale=neg_temp[:, 0:1],
                    accum_out=S[:, b:b + 1])

        # ---- finals: loss_row = Ln(S) + temp * pdist
        logz = singles.tile([128, QB], f32)
        nc.scalar.activation(out=logz, in_=S, func=AF.Ln, bias=0.0, scale=1.0)
        picked = singles.tile([128, QB], f32)
        nc.vector.tensor_scalar_mul(picked, pdist, pos_temp[:, 0:1])
        loss_t = singles.tile([128, QB], f32)
        nc.vector.tensor_add(loss_t, picked, logz)
        nc.sync.dma_start(out=out_d, in_=loss_t)


def build_program():
    nc = bacc.Bacc("TRN2", target_bir_lowering=False, debug=False,
                   num_devices=N_CORES)
    featT = nc.dram_tensor("featT", [D, NQ], bf16, kind="ExternalInput").ap()
    featn = nc.dram_tensor("featn", [NQ, D], f32, kind="ExternalInput").ap()
    feat2T = nc.dram_tensor("feat2T", [D, M], bf16, kind="ExternalInput").ap()
    sel = nc.dram_tensor("sel", [NQ, D], f32, kind="ExternalInput").ap()
    temp = nc.dram_tensor("temp", [1, 1], f32, kind="ExternalInput").ap()
    out = nc.dram_tensor("out", [128, QB], f32, kind="ExternalOutput").ap()
    with tile.TileContext(nc) as tc:
        _body(tc, out, featT, featn, feat2T, sel, temp)
    nc.compile()
    return nc


def make_in_maps(feat, feat2, temp, labels):
    feat = np.ascontiguousarray(np.asarray(feat, dtype=np.float32))
    feat2 = np.ascontiguousarray(np.asarray(feat2, dtype=np.float32))
    labels_np = np.asarray(labels).astype(np.int64)
    temp_np = np.asarray(temp, dtype=np.float32).reshape(1, 1)
    import ml_dtypes
    feat2T = np.ascontiguousarray(feat2.T).astype(ml_dtypes.bfloat16)
    sel_full = feat2[labels_np]
    in_maps = []
    for c in range(N_CORES):
        fs = feat[c * NQ:(c + 1) * NQ]
        in_maps.append({
            "featT": np.ascontiguousarray(fs.T).astype(ml_dtypes.bfloat16),
            "featn": fs,
            "feat2T": feat2T,
            "sel": np.ascontiguousarray(sel_full[c * NQ:(c + 1) * NQ]),
            "temp": temp_np,
        })
    return in_maps


def combine_outputs(per_core_outs):
    # out[p, b] is the loss for query q = b*128 + p of that core's shard
    rows = [np.asarray(o).T.reshape(-1) for o in per_core_outs]
    return np.float32(np.concatenate(rows).mean())


_PROGRAM = None


def kernel(feat, feat2, temp, labels):
    global _PROGRAM
    if _PROGRAM is None:
        _PROGRAM = build_program()
    in_maps = make_in_maps(feat, feat2, temp, labels)
    res = run_bass_kernel_spmd(_PROGRAM, in_maps, core_ids=list(range(N_CORES)))
    return combine_outputs([r["out"] for r in res.results])


# revision 22
# speedup vs baseline: 1.1854x; 1.0703x over previous
"""Trainium2 Bass kernel for nn_CLoss_68521908241007 (retrieval_knn).

Math (per the reference):
  sq_dist[i,j] = ||feat_i||^2 + ||feat2_j||^2 - 2 feat_i . feat2_j
  logits = -temp * sqrt(sq_dist)
  loss = mean_i( logsumexp_j(logits[i,:]) - logits[i, labels_i] )

Sharding: feat rows split across 8 cores (1024 queries each); feat2 replicated.
Each core computes its 1024x8192 block and returns per-row losses; the host
concatenates and takes the mean (the "all-reduce").

Per-core device pipeline:
  - PE (bf16): per 4-bank psum group, 4 fold matmuls (K=1, weights=-0.5*ones,
    rhs=bf16(y_sq-128)) then 4 main matmuls (featT.T @ feat2T) accumulate, so
    psum = G - 0.5*(y_sq-128).
  - ACT: dist = Sqrt(-2*psum + (x_sq+128))  [PSUM->SBUF, bf16 out]
         e    = Exp(-temp*dist)             [in place, fused row-sum accum]
  - All sqrts for all 8 q-blocks run before all exps: one activation-table
    load each (a switch costs ~2.7us).
  - y_sq via DVE square + PE ones-reduce; x_sq / picked distance via DVE.
"""

import numpy as np
from contextlib import ExitStack

import concourse.bass as bass
import concourse.bacc as bacc
import concourse.mybir as mybir
import concourse.tile as tile
from concourse.bass_utils import run_bass_kernel_spmd

AF = mybir.ActivationFunctionType
ALU = mybir.AluOpType
AX = mybir.AxisListType
f32 = mybir.dt.float32
bf16 = mybir.dt.bfloat16

N_CORES = 8
N, M, D = 8192, 8192, 128
NQ = N // N_CORES        # queries per core
QB = NQ // 128           # q-blocks per core (8)
KSEG = 512               # keys per matmul
NKSEG = M // KSEG        # 16
GRP = 4                  # k-segs per psum group (4 banks)
NGRP = NKSEG // GRP      # 4 groups per q-block


def _body(tc, out_d, featT_d, featn_d, feat2T_d, sel_d, temp_d):
    nc = tc.nc
    with ExitStack() as ctx:
        singles = ctx.enter_context(tc.tile_pool(name="singles", bufs=1))
        sqp = ctx.enter_context(tc.tile_pool(name="sqp", bufs=4))
        distp = ctx.enter_context(tc.tile_pool(name="distp", bufs=QB))
        psp = ctx.enter_context(tc.tile_pool(name="psp", bufs=2, space="PSUM"))
        smallp = ctx.enter_context(tc.tile_pool(name="smallp", bufs=2))

        # ---- inputs -> SBUF; small tensors first so they land early
        featn_sb = singles.tile([128, QB, D], f32)
        nc.sync.dma_start(out=featn_sb,
                          in_=featn_d.rearrange("(b p) d -> p b d", p=128))
        sel_sb = singles.tile([128, QB, D], f32)
        nc.sync.dma_start(out=sel_sb,
                          in_=sel_d.rearrange("(b p) d -> p b d", p=128))
        featT_sb = singles.tile([D, NQ], bf16)
        nc.sync.dma_start(out=featT_sb, in_=featT_d)
        pos_temp = singles.tile([128, 1], f32)
        nc.sync.dma_start(out=pos_temp, in_=temp_d.to_broadcast((128, 1)))
        # feat2T as 16 independent seg tiles -> per-seg dependencies
        f2segs = []
        for s in range(NKSEG):
            seg = singles.tile([D, KSEG], bf16, name=f"f2seg{s}")
            nc.sync.dma_start(out=seg, in_=feat2T_d[:, s * KSEG:(s + 1) * KSEG])
            f2segs.append(seg)

        # ---- constants
        neghalf_f = singles.tile([1, 128], f32)
        nc.vector.memset(neghalf_f, -0.5)
        neghalf = singles.tile([1, 128], bf16)
        nc.vector.tensor_copy(neghalf, neghalf_f)
        ones_col_f = singles.tile([D, 1], f32)
        nc.vector.memset(ones_col_f, 1.0)
        ones_col = singles.tile([D, 1], bf16)
        nc.vector.tensor_copy(ones_col, ones_col_f)
        neg_temp = singles.tile([128, 1], f32)
        nc.vector.tensor_scalar_mul(neg_temp, pos_temp, -1.0)

        # ---- x_sq (+128 shift) for the sqrt bias -- all DVE, off the ACT path
        x_sq = singles.tile([128, QB], f32)
        featsq = singles.tile([128, QB, D], f32)
        nc.vector.tensor_mul(featsq, featn_sb, featn_sb)
        for b in range(QB):
            nc.vector.reduce_sum(x_sq[:, b:b + 1], featsq[:, b, :], axis=AX.X)
        xb = singles.tile([128, QB], f32)
        nc.vector.tensor_scalar_add(xb, x_sq, 128.0)

        # ---- y_sq segs: bf16(colnorm(feat2T_seg) - 128) via DVE sq + PE ones-reduce
        ysegs = []
        for s in range(NKSEG):
            sq = sqp.tile([128, KSEG], bf16, tag="sq")
            nc.vector.tensor_mul(sq, f2segs[s], f2segs[s])
            ps_y = psp.tile([128, GRP * KSEG], f32, tag="ps")
            nc.tensor.matmul(ps_y[0:1, 0:KSEG], lhsT=ones_col,
                             rhs=sq, start=True, stop=True)
            yseg = singles.tile([1, KSEG], bf16, name=f"yseg{s}")
            nc.vector.tensor_scalar_add(yseg, ps_y[0:1, 0:KSEG], -128.0)
            ysegs.append(yseg)

        # ---- picked-label squared distance (DVE; only needed near the end)
        psq = singles.tile([128, QB], f32)
        sel2 = singles.tile([128, QB, D], f32)
        nc.vector.tensor_copy(sel2, sel_sb)
        diff_all = singles.tile([128, QB, D], f32)
        nc.vector.tensor_sub(diff_all, featn_sb, sel2)
        diffsq = singles.tile([128, QB, D], f32)
        nc.vector.tensor_mul(diffsq, diff_all, diff_all)
        for b in range(QB):
            nc.vector.reduce_sum(psq[:, b:b + 1], diffsq[:, b, :], axis=AX.X)

        # ---- main: all matmuls+sqrts (one table phase), then all exps
        S = singles.tile([128, QB], f32)
        pdist = singles.tile([128, QB], f32)
        dist_tiles = []
        for b in range(QB):
            dist_t = distp.tile([128, M], bf16, tag="dist")
            dist_tiles.append(dist_t)
            for g in range(NGRP):
                ps = psp.tile([128, GRP * KSEG], f32, tag="ps")
                # 4 folds back-to-back (one weight load), then 4 mains
                for si in range(GRP):
                    nc.tensor.matmul(
                        ps[:, si * KSEG:(si + 1) * KSEG], lhsT=neghalf,
                        rhs=ysegs[g * GRP + si], start=True, stop=False)
                for si in range(GRP):
                    nc.tensor.matmul(
                        ps[:, si * KSEG:(si + 1) * KSEG],
                        lhsT=featT_sb[:, b * 128:(b + 1) * 128],
                        rhs=f2segs[g * GRP + si], start=False, stop=True)
                nc.scalar.activation(
                    out=dist_t[:, g * GRP * KSEG:(g + 1) * GRP * KSEG],
                    in_=ps, func=AF.Sqrt, bias=xb[:, b:b + 1], scale=-2.0)
        # picked-label distance while the sqrt table is still loaded
        nc.scalar.activation(out=pdist, in_=psq, func=AF.Sqrt,
                             bias=0.0, scale=1.0)
        for b in range(QB):
            nc.scalar.activation(
                out=dist_tiles[b], in_=dist_tiles[b], func=AF.Exp,
                bias=0.0, scale=neg_temp[:, 0:1],
                accum_out=S[:, b:b + 1])

        # ---- finals: loss_row = Ln(S) + temp * pdist
        logz = singles.tile([128, QB], f32)
        nc.scalar.activation(out=logz, in_=S, func=AF.Ln, bias=0.0, scale=1.0)
        picked = singles.tile([128, QB], f32)
        nc.vector.tensor_scalar_mul(picked, pdist, pos_temp[:, 0:1])
        loss_t = singles.tile([128, QB], f32)
        nc.vector.tensor_add(loss_t, picked, logz)
        nc.sync.dma_start(out=out_d, in_=loss_t)


def build_program():
    nc = bacc.Bacc("TRN2", target_bir_lowering=False, debug=False,
                   num_devices=N_CORES)
    featT = nc.dram_tensor("featT", [D, NQ], bf16, kind="ExternalInput").ap()
    featn = nc.dram_tensor("featn", [NQ, D], f32, kind="ExternalInput").ap()
    feat2T = nc.dram_tensor("feat2T", [D, M], bf16, kind="ExternalInput").ap()
    sel = nc.dram_tensor("sel", [NQ, D], f32, kind="ExternalInput").ap()
    temp = nc.dram_tensor("temp", [1, 1], f32, kind="ExternalInput").ap()
    out = nc.dram_tensor("out", [128, QB], f32, kind="ExternalOutput").ap()
    with tile.TileContext(nc) as tc:
        _body(tc, out, featT, featn, feat2T, sel, temp)
    nc.compile()
    return nc


def make_in_maps(feat, feat2, temp, labels):
    import ml_dtypes
    feat = np.ascontiguousarray(np.asarray(feat, dtype=np.float32))
    feat2 = np.ascontiguousarray(np.asarray(feat2, dtype=np.float32))
    labels_np = np.asarray(labels).astype(np.int64)
    temp_np = np.asarray(temp, dtype=np.float32).reshape(1, 1)
    feat2T = np.ascontiguousarray(feat2.T).astype(ml_dtypes.bfloat16)
    sel_full = feat2[labels_np]
    in_maps = []
    for c in range(N_CORES):
        fs = feat[c * NQ:(c + 1) * NQ]
        in_maps.append({
            "featT": np.ascontiguousarray(fs.T).astype(ml_dtypes.bfloat16),
            "featn": fs,
            "feat2T": feat2T,
            "sel": np.ascontiguousarray(sel_full[c * NQ:(c + 1) * NQ]),
            "temp": temp_np,
        })
    return in_maps


def combine_outputs(per_core_outs):
    # out[p, b] is the loss for query q = b*128 + p of that core's shard
    rows = [np.asarray(o).T.reshape(-1) for o in per_core_outs]
    return np.float32(np.concatenate(rows).mean())


_PROGRAM = None


def kernel(feat, feat2, temp, labels):
    global _PROGRAM
    if _PROGRAM is None:
        _PROGRAM = build_program()
    in_maps = make_in_maps(feat, feat2, temp, labels)
    res = run_bass_kernel_spmd(_PROGRAM, in_maps, core_ids=list(range(N_CORES)))
    return combine_outputs([r["out"] for r in res.results])
